# revision 1
# baseline (speedup 1.0000x reference)
"""Trainium2 Bass kernel for nn_DiscreteMMSE (raw bass, manual scheduling).

Reference computation (per batch b):
    proj[n,t] = data[b,n,:] @ W[:,t]
    logp      = -0.5*(targets - proj)^2 + const
    csum      = cumsum_n(logp);  alpha = softmax_t(csum[n-1])
    pred[n]   = sum_t alpha[n-1,t] * proj[n,t]   (n>=1)
    pred[0]   = data[b,0,:] @ W.mean(axis=1)

Restructuring (validated in numpy):
  * err[n,t] = y[n] - x_n.w_t via ONE augmented matmul
        lhsT = [data_b^T ; y_b] (65,128),  rhs = [-W ; 1] (65,4096)
  * softmax const terms cancel; csum = L @ err^2 with
    L[m,n_out] = -0.5*(m < n_out): the strict inequality bakes in the
    "alpha uses the PREVIOUS point's csum" shift, so every downstream op is
    partition-aligned (no cross-partition shifts anywhere).
  * pred[n] = y[n] - (sum_t expw*err)/(sum_t expw): the division and the
    cross-chunk streaming-softmax rescale fold into per-partition scalars of
    one fused DVE op (scalar_tensor_tensor with accum_out).
  * task axis in 4 chunks of 1024: PSUM = err chunk (2 banks) x2 + csum
    chunk (2 banks) x2 = exactly 8 banks, double buffered.
  * err is recomputed in fp16 for the final weighted dot (PE columns are
    cheaper than a PSUM->SBUF copy pass on DVE/ACT); expw is stored fp16
    (rel-err ~3e-4 vs fp32 reference; bf16 gave ~2.4e-3).
  * the per-batch softmax-combine (global max, denominators, division) is
    deferred to one short batched tail over (128, 64)-column state, so the
    steady-state loop is a pure 3-engine pipeline (PE err/csum/late matmuls,
    ACT square/exp, DVE max/weighted-sum), ~377us/core in the cost-model
    timeline (from 542us for the naive schedule).

Raw bass with explicit semaphores because this toolchain's walrus rejects
instructions carrying >1 semaphore wait (Tile's scheduler emits multi-wait
sync_info). A tiny planner tracks per-engine counters, resolves every
cross-engine dependency to a single wait_ge threshold, and elides waits
already implied by program order.

Sharded batch-parallel over 8 cores: 16 batches/core, W replicated.
"""

from contextlib import ExitStack

import numpy as np

import concourse.bass as bass
from concourse import mybir
from concourse.bass_utils import run_bass_kernel_spmd

B, N, D, T = 128, 128, 64, 4096
NCORES = 8
BS = B // NCORES          # batches per core
CW = 1024                 # task-axis chunk width (2 PSUM banks fp32)
NQ = T // CW
MM = 512                  # one PSUM bank of fp32 / max fp32 moving dim
NK = BS * NQ              # chunk count

F32 = mybir.dt.float32
F32R = mybir.dt.float32r
BF16 = mybir.dt.bfloat16
F16 = mybir.dt.float16
AX = mybir.AxisListType.X
OP = mybir.AluOpType
AF = mybir.ActivationFunctionType

# dtype config for the three matmul passes ('f32' | 'f32r'), late also 'f16'
import os
ERR_DT = os.environ.get("ERR_DT", "f16x3")
CSUM_DT = os.environ.get("CSUM_DT", "f32")
LATE_DT = os.environ.get("LATE_DT", "f16")


def _v(ap, dt):
    # operand tiles are declared with the f32r dtype directly (walrus requires
    # producers of f32r-matmul inputs to round on write), so this is identity
    return ap


class Planner:
    """Records per-engine step lists with resolved single-sem wait thresholds."""

    def __init__(self):
        self.steps = {"PE": [], "ACT": [], "DVE": [], "POOL": []}
        self.counts = {"PE": 0, "ACT": 0, "DVE": 0, "POOL": 0,
                       "din": 0, "dout": 0}
        self.waited = {e: {} for e in self.steps}

    def step(self, eng, emit, waits=(), inc=None):
        """inc: (sem_name, value) or None -> defaults to (engine sem, 1)."""
        waits = list(waits)
        if eng == "DVE":
            # DVE's per-op DRAIN makes each op complete (and fire its sem inc)
            # before the next issues, so a self-wait at the previous count is
            # free at runtime -- but it gives the sim's race detector the
            # sem-based happens-before it demands for same-engine hazards.
            prev = self.counts.get("dve", 0)
            if prev > 0:
                waits.insert(0, ("dve", prev))
        real = []
        for sem_name, thr in waits:
            if thr is None or thr <= 0:
                continue
            if self.waited[eng].get(sem_name, 0) >= thr:
                continue
            self.waited[eng][sem_name] = thr
            real.append((sem_name, thr))
        if inc is None:
            inc = (eng.lower() if eng != "POOL" else "pool", 1)
        if inc is not False:
            self.counts.setdefault(inc[0], 0)
            self.counts[inc[0]] += inc[1]
        self.steps[eng].append((emit, real, inc if inc is not False else None))
        return self.counts[inc[0]] if inc is not False else None


def build_nc():
    nc = bass.Bass("TRN2")
    ctx = ExitStack()

    data_h = nc.dram_tensor("data_s", [BS, N, D], F32, kind="ExternalInput")
    tgt_h = nc.dram_tensor("targets_s", [BS, N], F32, kind="ExternalInput")
    w_h = nc.dram_tensor("W", [D, T], F32, kind="ExternalInput")
    out_h = nc.dram_tensor("out_s", [BS, N], F32, kind="ExternalOutput")
    ident_h = nc.inline_tensor(np.eye(128, dtype=np.float32), name="ident128")
    lmat = (-0.5 * np.tril(np.ones((N, N), np.float32), -1).T).copy()
    l_h = nc.inline_tensor(lmat.astype(np.float32), name="lmat")

    def sb(name, shape, dt):
        return ctx.enter_context(nc.sbuf_tensor(name, shape, dt))

    def ps(name, shape, dt):
        return ctx.enter_context(nc.psum_tensor(name, shape, dt))

    ERR_F = F32R if ERR_DT == "f32r" else F32
    CS_F = F32R if CSUM_DT == "f32r" else F32
    w_sb = sb("w_sb", [D, T], F32)
    ident = sb("ident", [128, 128], F32)
    l_sb = sb("l_sb", [N, N], CS_F)
    rhs_f32 = sb("rhs_f32", [D + 1, T], ERR_F)
    rhs_bf = sb("rhs_bf", [D + 1, T], F16)
    data_nat = sb("data_nat", [N, BS * D], F32)
    tgt_nat = sb("tgt_nat", [BS, N], F32)
    lhsT_f32 = sb("lhsT_f32", [D + 1, BS * N], ERR_F)
    lhsT_bf = sb("lhsT_bf", [D + 1, BS * N], F16)
    lhsT_lo = sb("lhsT_lo", [D + 1, BS * N], F16)
    rhs_lo = sb("rhs_lo", [D + 1, T], F16)
    tgtT = sb("tgtT", [N, BS], F32)
    preds = sb("preds", [N, BS], F32)
    out_nat = sb("out_nat", [BS, N], F32)
    pred0_sb = sb("pred0_sb", [1, BS], F32)
    wsum = sb("wsum", [D, 1], ERR_F)
    zcol = sb("zcol", [128, 1], F32)
    err2_sb = [sb(f"err2_{i}", [N, CW], CS_F) for i in range(3)]
    expw = [sb(f"expw_{i}", [N, CW], F16) for i in range(NQ)]
    scr = [sb(f"scr_{i}", [N, CW], F32) for i in range(2)]  # fp16 out + scalar-AP stt mis-accumulates on HW
    negMq = sb("negMq", [N, NK], F32)
    dq_all = sb("dq_all", [N, NK], F32)
    nq_all = sb("nq_all", [N, NK], F32)
    cq_all = sb("cq_all", [N, NK], F32)
    prod = sb("prod", [N, NK], F32)
    negMg_t = sb("negMg_t", [N, BS], F32)
    Dall = sb("Dall", [N, BS], F32)
    rDall = sb("rDall", [N, BS], F32)
    Sraw = sb("Sraw", [N, BS], F32)
    SSml = sb("SSml", [N, BS], F32)

    err_ps = [ps(f"err_ps_{i}", [N, CW], F32) for i in range(2)]
    csum_ps = [ps(f"csum_ps_{i}", [N, CW], F32) for i in range(2)]

    # setup-phase views into main psum tensors (distinct banks where concurrent)
    tp_view = lambda b: err_ps[0][0:D, (b % 2) * MM:(b % 2) * MM + N]
    tgtT_view = err_ps[1][0:N, 0:BS]
    p0_view = err_ps[1][0:1, MM:MM + BS]
    tout_view = err_ps[0][0:BS, 0:N]

    if LATE_DT in ("bf16", "f16"):
        lhsT_late, rhs_late = lhsT_bf, rhs_bf
    else:
        lhsT_late, rhs_late = lhsT_f32, rhs_f32

    P = Planner()

    # ---------------- DMAs (gpsimd / SWDGE) ----------------
    def dma_data_chunk(j):
        bs = slice(j * 4, (j + 1) * 4)
        return nc.gpsimd.dma_start(
            out=data_nat[:, j * 4 * D:(j + 1) * 4 * D].rearrange(
                "n (b d) -> n b d", d=D),
            in_=data_h[bs].rearrange("b n d -> n b d"))

    # one sem per setup DMA: SWDGE queues complete out of order, so partial
    # thresholds on one shared sem are ambiguous happens-before
    dmas = [
        ("di", lambda: nc.gpsimd.dma_start(out=ident[:], in_=ident_h[:])),
        ("dc0", lambda: dma_data_chunk(0)),
        ("dc1", lambda: dma_data_chunk(1)),
        ("dc2", lambda: dma_data_chunk(2)),
        ("dc3", lambda: dma_data_chunk(3)),
        ("dw", lambda: nc.gpsimd.dma_start(out=w_sb[:], in_=w_h[:])),
        ("dl", lambda: nc.gpsimd.dma_start(out=l_sb[:], in_=l_h[:])),
        ("dt", lambda: nc.gpsimd.dma_start(out=tgt_nat[:], in_=tgt_h[:])),
        ("dy", lambda: nc.gpsimd.dma_start(
            out=lhsT_f32[D:D + 1, :],
            in_=tgt_h[:].rearrange("b n -> (b n)")[None, :])),
    ]
    for s, d in dmas:
        P.step("POOL", d, inc=(s, 16))

    # ---------------- setup: transposes + operand prep ----------------
    tC = {}
    tT = {}
    for b in range(BS):
        w = [("di", 16), (f"dc{b // 4}", 16)]
        if b >= 2:
            w.append(("dve", tC[b - 2]))
        tT[b] = P.step("PE", (lambda b=b: nc.tensor.transpose(
            tp_view(b), data_nat[:, b * D:(b + 1) * D], ident[:])), w)
        tC[b] = P.step("DVE", (lambda b=b: nc.vector.tensor_copy(
            out=lhsT_f32[0:D, b * N:(b + 1) * N], in_=tp_view(b))),
            [("pe", tT[b])])
    e_ttgt = P.step("PE", lambda: nc.tensor.transpose(
        tgtT_view, tgt_nat[:], ident[0:BS, 0:BS]),
        [("dve", tC[BS - 2]), ("dt", 16)])
    e_ctgt = P.step("DVE", lambda: nc.vector.tensor_copy(
        out=tgtT[:], in_=tgtT_view), [("pe", e_ttgt)])
    P.step("DVE", lambda: nc.vector.tensor_scalar_mul(
        out=rhs_f32[0:D, :], in0=w_sb[:], scalar1=-1.0), [("dw", 16)])
    P.step("DVE", lambda: nc.vector.memset(rhs_f32[D:D + 1, :], 1.0))
    P.step("DVE", lambda: nc.vector.tensor_copy(
        out=lhsT_bf[:], in_=lhsT_f32[:]), [("dy", 16)])
    P.step("DVE", lambda: nc.vector.tensor_copy(out=rhs_bf[:], in_=rhs_f32[:]))
    if ERR_DT == "f16x3":
        P.step("DVE", lambda: nc.vector.tensor_sub(
            out=lhsT_lo[:], in0=lhsT_f32[:], in1=lhsT_bf[:]))
        P.step("DVE", lambda: nc.vector.tensor_sub(
            out=rhs_lo[:], in0=rhs_f32[:], in1=rhs_bf[:]))
    P.step("DVE", lambda: nc.vector.reduce_sum(out=wsum[:], in_=w_sb[:], axis=AX))
    e_wscale = P.step("DVE", lambda: nc.vector.tensor_scalar_mul(
        out=wsum[:], in0=wsum[:], scalar1=1.0 / T))
    e_zcol = P.step("DVE", lambda: nc.vector.memset(zcol[:], 0.0))
    d0 = lhsT_f32[0:D, :].rearrange("d (b n) -> d b n", n=N)[:, :, 0:1]
    e_p0 = P.step("PE", lambda: nc.tensor.matmul(
        p0_view, wsum[:], d0, start=True, stop=True), [("dve", e_wscale)])
    e_p0c = P.step("DVE", lambda: nc.vector.tensor_copy(
        out=pred0_sb[:], in_=p0_view), [("pe", e_p0)])

    # ---------------- main loop (software-pipelined emit order) ----------------
    # e-slots (err_ps[k%2]) hold err chunks; c-slots (csum_ps[.]) hold csum
    # chunks AND the late err recompute. PE emits err one chunk ahead of csum;
    # ACT emits Sq one chunk ahead of exp. The main loop is a pure
    # err->Sq->csum->max->exp / late->stt pipeline; ALL per-batch softmax
    # combine state lives in per-(b,q) columns and is folded in a short
    # batched tail, so no engine ever blocks on a cross-engine round trip.
    t_err, t_Sq, t_csum, t_max, t_exp, t_late, t_stt = {}, {}, {}, {}, {}, {}, {}
    e_stt = t_stt

    def emit_mm(dst, lhs, rhs_all, q, dt):
        last = None
        for j in range(CW // MM):
            c0 = q * CW + j * MM
            last = nc.tensor.matmul(
                dst[:, j * MM:(j + 1) * MM], lhs,
                rhs_all[:, c0:c0 + MM], start=True, stop=True)
        return last

    def emit_err_f16x3(b, q, es):
        bs = slice(b * N, (b + 1) * N)
        last = None
        for j in range(CW // MM):
            c0 = q * CW + j * MM
            dst = err_ps[es][:, j * MM:(j + 1) * MM]
            nc.tensor.matmul(dst, lhsT_bf[:, bs], rhs_bf[:, c0:c0 + MM],
                             start=True, stop=False)
            nc.tensor.matmul(dst, lhsT_bf[:, bs], rhs_lo[:, c0:c0 + MM],
                             start=False, stop=False)
            last = nc.tensor.matmul(dst, lhsT_lo[:, bs],
                                    rhs_bf[:, c0:c0 + MM],
                                    start=False, stop=True)
        return last

    def pe_err(k):
        b, q = divmod(k, NQ)
        es = k % 2
        w = []
        if k < 2:
            w.append(("dve", e_p0c))
        else:
            w.append(("act", t_Sq[k - 2]))
        if ERR_DT == "f16x3":
            t_err[k] = P.step("PE", (lambda b=b, q=q, es=es:
                                     emit_err_f16x3(b, q, es)), w)
        else:
            t_err[k] = P.step("PE", (lambda b=b, q=q, es=es: emit_mm(
                err_ps[es], lhsT_f32[:, b * N:(b + 1) * N], rhs_f32, q,
                ERR_DT)), w)

    def pe_csum(k):
        b, q = divmod(k, NQ)
        cs = k % 2
        w = [("act", t_Sq[k])]
        if k < 2:
            w.append(("dl", 16))
        if q < 2:
            if b >= 1:
                w.append(("dve", t_stt[(b - 1) * NQ + 2 + q]))
        else:
            w.append(("dve", t_max[k - 2]))
        t_csum[k] = P.step("PE", (lambda k=k, cs=cs: emit_mm(
            csum_ps[cs], l_sb[:], err2_sb[k % 3], 0, CSUM_DT)), w)

    def pe_late(b, q):
        k = b * NQ + q
        cs = q % 2
        if q < 2:
            w = [("act", t_exp[4 * b + 2 + q]), ("dve", t_max[4 * b + 2 + q])]
        else:
            w = [("dve", t_stt[k - 2])]
        t_late[k] = P.step("PE", (lambda b=b, q=q, cs=cs: emit_mm(
            csum_ps[cs], lhsT_late[:, b * N:(b + 1) * N], rhs_late, q,
            LATE_DT)), w)

    def act_sq(k):
        es = k % 2
        t_Sq[k] = P.step("ACT", (lambda k=k, es=es: nc.scalar.activation(
            out=err2_sb[k % 3][:], in_=err_ps[es][:], func=AF.Square,
            bias=zcol[:], scale=1.0)),
            [("pe", t_err[k]), ("dve", e_zcol)])

    def act_exp(k):
        q = k % NQ
        cs = k % 2
        t_exp[k] = P.step("ACT", (lambda k=k, cs=cs, q=q:
                                  nc.scalar.activation(
            out=expw[q][:], in_=csum_ps[cs][:], func=AF.Exp,
            bias=negMq[:, k:k + 1], scale=1.0,
            accum_out=dq_all[:, k:k + 1])),
            [("dve", t_max[k]), ("pe", t_csum[k])])

    def dve_max(k):
        cs = k % 2
        t_max[k] = P.step("DVE", (lambda k=k, cs=cs:
                                  nc.vector.tensor_reduce(
            out=negMq[:, k:k + 1], in_=csum_ps[cs][:],
            axis=AX, op=OP.max, negate=True)), [("pe", t_csum[k])])

    def dve_stt(b, q):
        k = b * NQ + q
        cs = q % 2
        t_stt[k] = P.step("DVE", (lambda k=k, q=q, cs=cs:
                                  nc.vector.scalar_tensor_tensor(
            out=scr[q % 2][:], in0=csum_ps[cs][:],
            scalar=1.0, in1=expw[q][:],
            op0=OP.mult, op1=OP.mult,
            accum_out=nq_all[:, k:k + 1])),
            [("pe", t_late[k]), ("act", t_exp[k])])

    # PE stream
    pe_q = []
    for b in range(BS):
        k0 = b * NQ
        if b == 0:
            pe_q += [("err", 0), ("err", 1)]
        pe_q += [("csum", k0), ("err", k0 + 2), ("csum", k0 + 1),
                 ("err", k0 + 3), ("csum", k0 + 2)]
        if b < BS - 1:
            pe_q += [("err", k0 + 4), ("err", k0 + 5)]
        pe_q += [("csum", k0 + 3),
                 ("late", k0 + 0), ("late", k0 + 1),
                 ("late", k0 + 2), ("late", k0 + 3)]

    act_q = [("sq", 0)]
    for k in range(NK):
        if k + 1 < NK:
            act_q.append(("sq", k + 1))
        act_q.append(("exp", k))

    dve_q = []
    for b in range(BS):
        k0 = b * NQ
        dve_q += [("max", k0), ("max", k0 + 1), ("max", k0 + 2),
                  ("max", k0 + 3),
                  ("stt", k0 + 0), ("stt", k0 + 1), ("stt", k0 + 2),
                  ("stt", k0 + 3)]

    def deps_ready(item):
        kind, a = item
        if kind == "err":
            return a < 2 or (a - 2) in t_Sq
        if kind == "csum":
            b, q = divmod(a, NQ)
            if a not in t_Sq:
                return False
            if q < 2:
                return b == 0 or ((b - 1) * NQ + 2 + q) in t_stt
            return (a - 2) in t_max
        if kind == "late":
            b, q = divmod(a, NQ)
            if q < 2:
                return (4 * b + 2 + q) in t_exp
            return (a - 2) in t_stt
        if kind == "sq":
            return a in t_err
        if kind == "exp":
            return a in t_max and a in t_csum
        if kind == "max":
            return a in t_csum
        if kind == "stt":
            return a in t_late and a in t_exp
        raise ValueError(kind)

    emitters = {
        "err": pe_err, "csum": pe_csum, "late": pe_late,
        "sq": act_sq, "exp": act_exp, "max": dve_max, "stt": dve_stt,
    }
    queues = [pe_q, act_q, dve_q]
    idx = [0, 0, 0]
    while any(i < len(q) for i, q in zip(idx, queues)):
        progressed = False
        for qi, q in enumerate(queues):
            while idx[qi] < len(q) and deps_ready(q[idx[qi]]):
                kind, a = q[idx[qi]]
                if kind in ("late", "stt"):
                    emitters[kind](a // NQ, a % NQ)
                else:
                    emitters[kind](a)
                idx[qi] += 1
                progressed = True
        if not progressed:
            raise RuntimeError("plan deadlock")

    # ---------------- batched softmax-combine tail ----------------
    # negMq[:, k] = -M_{b,q}; global per-b: negMg = min_q(-M_q) = -M_b
    # cq = exp(M_q - M_b); D_b = sum_q cq*dq; S_b = (sum_q cq*nq)/D_b
    # pred = y - S  (all (128, BS*NQ)/(128, BS) sized ops)
    P.step("DVE", lambda: nc.vector.tensor_reduce(
        out=negMg_t[:], in_=negMq[:].rearrange("p (b q) -> p b q", q=NQ),
        axis=AX, op=OP.min), [])
    e_cq = None
    for b in range(BS):
        e_cq = P.step("ACT", (lambda b=b: nc.scalar.activation(
            out=cq_all[:, b * NQ:(b + 1) * NQ],
            in_=negMq[:, b * NQ:(b + 1) * NQ], func=AF.Exp,
            bias=negMg_t[:, b:b + 1], scale=-1.0)),
            [("dve", P.counts["dve"])])
    P.step("DVE", lambda: nc.vector.tensor_mul(
        out=prod[:], in0=cq_all[:], in1=dq_all[:]),
        [("act", e_cq)])
    P.step("DVE", lambda: nc.vector.tensor_reduce(
        out=Dall[:], in_=prod[:].rearrange("p (b q) -> p b q", q=NQ),
        axis=AX, op=OP.add), [])
    P.step("DVE", lambda: nc.vector.reciprocal(out=rDall[:], in_=Dall[:]), [])
    P.step("DVE", lambda: nc.vector.tensor_mul(
        out=prod[:], in0=cq_all[:], in1=nq_all[:]), [])
    P.step("DVE", lambda: nc.vector.tensor_reduce(
        out=Sraw[:], in_=prod[:].rearrange("p (b q) -> p b q", q=NQ),
        axis=AX, op=OP.add), [])
    P.step("DVE", lambda: nc.vector.tensor_mul(
        out=SSml[:], in0=Sraw[:], in1=rDall[:]), [])
    P.step("DVE", lambda: nc.vector.tensor_sub(
        out=preds[:], in0=tgtT[:], in1=SSml[:]), [])

    # ---------------- tail: assemble + store output ----------------
    e_p0w = P.step("DVE", lambda: nc.vector.tensor_copy(
        out=preds[0:1, :], in_=pred0_sb[:]), [])
    e_tout = P.step("PE", lambda: nc.tensor.transpose(
        tout_view, preds[:], ident[:]),
        [("dve", e_p0w), ("act", t_Sq[NK - 2])])
    e_outc = P.step("DVE", lambda: nc.vector.tensor_copy(
        out=out_nat[:], in_=tout_view), [("pe", e_tout)])
    P.step("POOL", lambda: nc.gpsimd.dma_start(out=out_h[:], in_=out_nat[:]),
           [("dve", e_outc)], inc=("dout", 16))
    P.step("POOL", None, [("dout", 16)], inc=False)

    # ---------------- emit ----------------
    with ctx:
        sems = {}
        for name in ("pe", "act", "dve", "dout", "di", "dw", "dl", "dt",
                     "dy", "dc0", "dc1", "dc2", "dc3"):
            sems[name] = ctx.enter_context(nc.semaphore(name=f"sem_{name}"))

        def run(eng_name, engine):
            for emit, waits, inc in P.steps[eng_name]:
                for sem_name, thr in waits:
                    engine.wait_ge(sems[sem_name], thr)
                inst = emit() if emit is not None else None
                if inst is not None and inc is not None:
                    inst.then_inc(sems[inc[0]], inc[1])

        with nc.Block() as block:
            @block.gpsimd
            def _(eng):
                run("POOL", eng)

            @block.tensor
            def _(eng):
                run("PE", eng)

            @block.scalar
            def _(eng):
                run("ACT", eng)

            @block.vector
            def _(eng):
                run("DVE", eng)

    return nc


_NC = None


def _get_nc():
    global _NC
    if _NC is None:
        _NC = build_nc()
    return _NC


def kernel(data, targets, W, _trace=False, _tc=None):
    data = np.ascontiguousarray(np.asarray(data), dtype=np.float32)
    targets = np.ascontiguousarray(np.asarray(targets), dtype=np.float32)
    W = np.ascontiguousarray(np.asarray(W), dtype=np.float32)
    nc = _get_nc()
    in_maps = []
    for c in range(NCORES):
        sl = slice(c * BS, (c + 1) * BS)
        in_maps.append({
            "data_s": np.ascontiguousarray(data[sl]),
            "targets_s": np.ascontiguousarray(targets[sl]),
            "W": W,
        })
    kw = {}
    if _trace:
        kw = dict(trace=True, trace_cores=_tc if _tc is not None else [0])
    res = run_bass_kernel_spmd(nc, in_maps, core_ids=list(range(NCORES)), **kw)
    out = np.concatenate([r["out_s"] for r in res.results], axis=0)
    if _trace:
        return out, res
    return out


if __name__ == "__main__":
    rng = np.random.default_rng(0)
    data = rng.standard_normal((B, N, D), dtype=np.float32)
    targets = rng.standard_normal((B, N), dtype=np.float32)
    W = rng.standard_normal((D, T), dtype=np.float32)
    out = kernel(data, targets, W)
    print("out", out.shape, out.dtype, np.abs(out).mean())

